# revision 41
# baseline (speedup 1.0000x reference)
"""Trainium2 Bass kernel for nn_DecoderLayer (Mamba block + BitNet FFN).

Sharding: channel-parallel mamba (256 ch/core) -> AllReduce (xproj rows) ->
DVE tensor_tensor_scan over (d,n) lanes -> AllToAll (d-shard -> t-shard) ->
sequence-parallel out_proj + rmsnorm + BitNet FFN (host-prequantized ternary
weights as fp8, exact bf16xfp8 matmuls) -> each core emits its 256-token slice.

v2: bf16 conv/activations, fp8 ternary FFN weights, early weight prefetch,
chunked in_proj, redundant clips dropped, repeat=N support for timing.
"""
import numpy as np
import ml_dtypes

try:
    import jax
    jax.config.update("jax_compilation_cache_dir", "/root/jaxcache")
    jax.config.update("jax_persistent_cache_min_compile_time_secs", 1.0)
except Exception:
    pass

import concourse.bass as bass
import concourse.mybir as mybir
import concourse.tile as tile
from concourse import bacc
from concourse.bass_utils import run_bass_kernel_spmd

BF16 = mybir.dt.bfloat16
F32 = mybir.dt.float32
F32R = mybir.dt.float32r
FP8 = mybir.dt.float8e4
AF = mybir.ActivationFunctionType
OP = mybir.AluOpType

L, DM, DI, DS, DC, DTR, DFF = 2048, 1024, 2048, 16, 4, 64, 4096
EPS = 1e-6
NCORES = 8
DIC = DI // NCORES   # 256 channels per core
NDT = DIC // 128     # 2 d-tiles
LT = L // NCORES     # 256 tokens per core
NTT = LT // 128      # 2 token-tiles
MAGIC = 12582912.0   # 1.5*2^23: x+M-M == rint(x) for |x|<2^22

_NC_CACHE = {}


def _declare(nc):
    t = {}
    t["xT"] = nc.dram_tensor("xT", [DM, L], BF16, kind="ExternalInput")
    t["x_tok"] = nc.dram_tensor("x_tok", [LT, DM], F32, kind="ExternalInput")
    t["winT"] = nc.dram_tensor("winT", [DM, 2 * 128 * NDT], BF16, kind="ExternalInput")
    t["convw"] = nc.dram_tensor("convw", [DIC, DC], F32, kind="ExternalInput")
    t["convb"] = nc.dram_tensor("convb", [DIC, 1], F32, kind="ExternalInput")
    t["wxpT"] = nc.dram_tensor("wxpT", [DIC, 96], BF16, kind="ExternalInput")
    t["wdtT"] = nc.dram_tensor("wdtT", [DTR, DIC], BF16, kind="ExternalInput")
    t["bdt"] = nc.dram_tensor("bdt", [DIC, 1], F32, kind="ExternalInput")
    t["acol"] = nc.dram_tensor("acol", [DIC, DS], F32, kind="ExternalInput")
    t["dpv"] = nc.dram_tensor("dpv", [DIC, 1], F32, kind="ExternalInput")
    t["woutT"] = nc.dram_tensor("woutT", [DI, DM], BF16, kind="ExternalInput")
    t["n1w"] = nc.dram_tensor("n1w", [1, DM], F32, kind="ExternalInput")
    t["n2w"] = nc.dram_tensor("n2w", [1, DM], F32, kind="ExternalInput")
    t["w1qT"] = nc.dram_tensor("w1qT", [DM, DFF], FP8, kind="ExternalInput")
    t["w2qT"] = nc.dram_tensor("w2qT", [DFF, DM], FP8, kind="ExternalInput")
    t["out"] = nc.dram_tensor("out", [LT, DM], F32, kind="ExternalOutput")
    return t


def _emit(nc, tc, ctx, g1, g2, t):
    import contextlib
    RG = [list(range(NCORES))]
    xT = t["xT"]; x_tok = t["x_tok"]; winT = t["winT"]; convw = t["convw"]
    convb = t["convb"]; wxpT = t["wxpT"]; wdtT = t["wdtT"]; bdt = t["bdt"]
    acol = t["acol"]; dpv = t["dpv"]; woutT = t["woutT"]; n1w = t["n1w"]
    n2w = t["n2w"]; w1qT = t["w1qT"]; w2qT = t["w2qT"]; out_t = t["out"]

    singles = ctx.enter_context(tc.tile_pool(name="singles", bufs=1))
    dram = ctx.enter_context(tc.tile_pool(name="dram", bufs=1, space="DRAM"))
    wpool = ctx.enter_context(tc.tile_pool(name="wpool", bufs=1))
    psA_stack = contextlib.ExitStack()
    psum_small = psA_stack.enter_context(
        tc.tile_pool(name="psA", bufs=3, space="PSUM"))
    act_stack = contextlib.ExitStack()
    actpool = act_stack.enter_context(tc.tile_pool(name="acts", bufs=1))

    # ---- small per-partition constants
    convw_sb, convb_sb, bdt_sb, acol_sb, dp_sb = [], [], [], [], []
    for dt in range(NDT):
        sl = slice(dt * 128, (dt + 1) * 128)
        t1 = singles.tile([128, DC], F32, name=f"cw{dt}")
        nc.sync.dma_start(t1[:, :], convw[sl, :])
        convw_sb.append(t1)
        t2 = singles.tile([128, 1], F32, name=f"cb{dt}")
        nc.sync.dma_start(t2[:, :], convb[sl, :])
        convb_sb.append(t2)
        t3 = singles.tile([128, 1], F32, name=f"bd{dt}")
        nc.sync.dma_start(t3[:, :], bdt[sl, :])
        bdt_sb.append(t3)
        t4 = singles.tile([128, DS], F32, name=f"ac{dt}")
        nc.sync.dma_start(t4[:, :], acol[sl, :])
        acol_sb.append(t4)
        t5 = singles.tile([128, 1], F32, name=f"dp{dt}")
        nc.sync.dma_start(t5[:, :], dpv[sl, :])
        dp_sb.append(t5)
    wxpT_sb = singles.tile([128, NDT, 96], BF16)
    nc.sync.dma_start(wxpT_sb[:, :, :],
                      wxpT.rearrange("(k p) m -> p k m", p=128))
    wdtT_sb = singles.tile([DTR, DIC], BF16)
    nc.sync.dma_start(wdtT_sb[:, :], wdtT[:, :])
    ident_bf = singles.tile([128, 128], BF16)
    from concourse.masks import make_identity
    make_identity(nc, ident_bf[:, :])
    dpdiag = []
    for dt in range(NDT):
        d = singles.tile([128, 128], BF16, name=f"dpd{dt}")
        nc.vector.tensor_scalar_mul(d[:, :], ident_bf[:, :], dp_sb[dt][:, 0:1])
        dpdiag.append(d)

    # ================= PHASE A: in_proj (channel-parallel) =================
    conv_stack = contextlib.ExitStack()
    convpool = conv_stack.enter_context(tc.tile_pool(name="convp", bufs=1))
    NXC = 4                    # xT chunks along L
    XC = L // NXC
    with tc.tile_pool(name="init", bufs=1) as init_pool:
        winT_sb = init_pool.tile([128, 8, 2 * 128 * NDT], BF16)
        nc.sync.dma_start(winT_sb[:, :, :],
                          winT.rearrange("(k p) m -> p k m", p=128))
        xT_re = xT.rearrange("(k p) l -> p k l", p=128)
        xT_c = []
        for c in range(NXC):
            xc = init_pool.tile([128, 8, XC], BF16, name=f"xc{c}")
            nc.sync.dma_start(xc[:, :, :], xT_re[:, :, c * XC:(c + 1) * XC])
            xT_c.append(xc)

        # ---- prefetch phase-B weights (queue behind critical loads)
        woutT_sb = wpool.tile([128, DI // 128, DM], BF16)
        nc.sync.dma_start(woutT_sb[:, :, :],
                          woutT.rearrange("(k p) m -> p k m", p=128))
        w1qT_sb = wpool.tile([128, 8, DFF], FP8)
        nc.sync.dma_start(w1qT_sb[:, :, :],
                          w1qT.rearrange("(k p) j -> p k j", p=128))
        w2qT_sb = wpool.tile([128, DFF // 128, DM], FP8)
        nc.sync.dma_start(w2qT_sb[:, :, :],
                          w2qT.rearrange("(k p) m -> p k m", p=128))

        u_pad, zs = [], []
        for dt in range(NDT):
            up = convpool.tile([128, L + 3], BF16, name=f"upad{dt}")
            nc.vector.memset(up[:, 0:3], 0.0)
            u_pad.append(up)
            zs.append(actpool.tile([128, L], BF16, name=f"zs{dt}"))

        # m-tiles: 0..NDT-1 are u chunks, NDT..2*NDT-1 are z chunks
        for c in range(NXC):
            for mt in range(2 * NDT):
                for cc in range(XC // 512):
                    ps = psum_small.tile([128, 512], F32, tag="psA")
                    lo = c * XC + cc * 512
                    for k in range(8):
                        nc.tensor.matmul(
                            ps[:, :],
                            winT_sb[:, k, mt * 128:(mt + 1) * 128],
                            xT_c[c][:, k, cc * 512:(cc + 1) * 512],
                            start=(k == 0), stop=(k == 7))
                    if mt < NDT:
                        nc.scalar.copy(u_pad[mt][:, 3 + lo: 3 + lo + 512],
                                       ps[:, :])
                    else:
                        nc.scalar.activation(
                            zs[mt - NDT][:, lo: lo + 512], ps[:, :], AF.Silu)

    # ================= conv + silu (bf16 chain) =================
    u_act = []
    for dt in range(NDT):
        ca = convpool.tile([128, L], BF16, name=f"cva{dt}", tag="cva")
        cb = convpool.tile([128, L], BF16, name=f"cvb{dt}", tag="cvb")
        nc.vector.tensor_scalar_mul(ca[:, :], u_pad[dt][:, 0:L],
                                    convw_sb[dt][:, 0:1])
        nc.vector.scalar_tensor_tensor(
            cb[:, :], u_pad[dt][:, 1:L + 1], convw_sb[dt][:, 1:2], ca[:, :],
            op0=OP.mult, op1=OP.add)
        nc.vector.scalar_tensor_tensor(
            ca[:, :], u_pad[dt][:, 2:L + 2], convw_sb[dt][:, 2:3], cb[:, :],
            op0=OP.mult, op1=OP.add)
        nc.vector.scalar_tensor_tensor(
            cb[:, :], u_pad[dt][:, 3:L + 3], convw_sb[dt][:, 3:4], ca[:, :],
            op0=OP.mult, op1=OP.add)
        ua = actpool.tile([128, L], BF16, name=f"uact{dt}")
        nc.scalar.activation(ua[:, :], cb[:, :], AF.Silu,
                             bias=convb_sb[dt][:, 0:1])
        u_act.append(ua)
    conv_stack.close()


    # ================= xproj partial + AllReduce =================
    xp_stack = contextlib.ExitStack()
    xppool = xp_stack.enter_context(tc.tile_pool(name="xpp", bufs=1))
    dbl_loc = xppool.tile([96, L], BF16)
    for c in range(L // 512):
        ps = psum_small.tile([96, 512], F32, tag="psA")
        for kt in range(NDT):
            nc.tensor.matmul(
                ps[:, :],
                wxpT_sb[:, kt, :],
                u_act[kt][:, c * 512:(c + 1) * 512],
                start=(kt == 0), stop=(kt == NDT - 1))
        nc.scalar.copy(dbl_loc[:, c * 512:(c + 1) * 512], ps[:, :])

    # bf16 AllReduce (halves the collective payload); B/C broadcasts read the
    # AR output in DRAM directly, no bounce copy needed
    ar_i = dram.tile([96, L], BF16)
    ar_o = dram.tile([96, L], BF16, addr_space="Shared")
    nc.sync.dma_start(ar_i[:, :], dbl_loc[:, :])
    nc.gpsimd.collective_compute("AllReduce", OP.add, replica_groups=RG,
                                 ins=[ar_i.opt()], outs=[ar_o.opt()])
    dbl_bf = xppool.tile([64, L], BF16)
    nc.sync.dma_start(dbl_bf[:, :], ar_o[0:DTR, :])
    bcb = ar_o

    # ================= delta = softplus(wdt @ dt + bdt) =================
    # all Exp ops batched before the Ln ops: fewer act-table reloads
    delta = []
    for dt in range(NDT):
        dl = actpool.tile([128, L], BF16, name=f"delta{dt}")
        for c in range(L // 512):
            ps = psum_small.tile([128, 512], F32, tag="psA")
            nc.tensor.matmul(
                ps[:, :],
                wdtT_sb[:, dt * 128:(dt + 1) * 128],
                dbl_bf[0:DTR, c * 512:(c + 1) * 512],
                start=True, stop=True)
            # exp(x + bdt) from PSUM, then ln(1+e) in-place later
            nc.scalar.activation(dl[:, c * 512:(c + 1) * 512], ps[:, :],
                                 AF.Exp, bias=bdt_sb[dt][:, 0:1])
        delta.append(dl)
    for dt in range(NDT):
        nc.scalar.activation(delta[dt][:, :], delta[dt][:, :], AF.Ln, bias=1.0)

    xp_stack.close()
    # delta*u in bf16 for the scan input product
    du_bf = []
    for dt in range(NDT):
        db = actpool.tile([128, L], BF16, name=f"dubf{dt}")
        nc.vector.tensor_tensor(db[:, :], delta[dt][:, :], u_act[dt][:, :],
                                op=OP.mult)
        du_bf.append(db)

    # ================= scan over n (16 states) =================
    psA_stack.close()
    yps_stack = contextlib.ExitStack()
    y_ps_pool = yps_stack.enter_context(
        tc.tile_pool(name="yps", bufs=1, space="PSUM"))
    y_ps = [y_ps_pool.tile([128, L], F32, name=f"yps{dt}") for dt in range(NDT)]

    scanp = act_stack.enter_context(tc.tile_pool(name="scanp", bufs=2))
    repp = act_stack.enter_context(tc.tile_pool(name="repp", bufs=3))
    for n in range(DS):
        brep = repp.tile([128, L], BF16, name=f"brep{n}", tag="brep")
        b_src = bcb[DTR + n:DTR + n + 1, :]
        nc.sync.dma_start(brep[:, :], bass.AP(
            tensor=b_src.tensor, offset=b_src.offset,
            ap=[[0, 128]] + [list(p) for p in b_src.ap[1:]]))
        crep = repp.tile([128, L], BF16, name=f"crep{n}", tag="crep")
        c_src = bcb[DTR + DS + n:DTR + DS + n + 1, :]
        nc.sync.dma_start(crep[:, :], bass.AP(
            tensor=c_src.tensor, offset=c_src.offset,
            ap=[[0, 128]] + [list(p) for p in c_src.ap[1:]]))
        for dt in range(NDT):
            dA = scanp.tile([128, L], BF16, name=f"dA{n}_{dt}", tag="dA")
            nc.scalar.activation(dA[:, :], delta[dt][:, :], AF.Exp,
                                 scale=acol_sb[dt][:, n:n + 1])
            dBu = scanp.tile([128, L], BF16, name=f"dBu{n}_{dt}", tag="dBu")
            nc.vector.tensor_tensor(dBu[:, :], du_bf[dt][:, :], brep,
                                    op=OP.mult)
            h = scanp.tile([128, L], BF16, name=f"h{n}_{dt}", tag="h")
            nc.vector.tensor_tensor_scan(h[:, :], dA[:, :], dBu[:, :], 0.0,
                                         OP.mult, OP.add)
            yt = scanp.tile([128, L], BF16, name=f"yt{n}_{dt}", tag="yt")
            nc.vector.tensor_tensor(yt[:, :], h[:, :], crep, op=OP.mult)
            for c in range(L // 512):
                nc.tensor.matmul(
                    y_ps[dt][:, c * 512:(c + 1) * 512],
                    ident_bf[:, :],
                    yt[:, c * 512:(c + 1) * 512],
                    start=(n == 0), stop=False,
                    skip_group_check=True)
    # final accumulation term: Dp * u via a diagonal stationary matrix
    for dt in range(NDT):
        for c in range(L // 512):
            nc.tensor.matmul(
                y_ps[dt][:, c * 512:(c + 1) * 512],
                dpdiag[dt][:, :],
                u_act[dt][:, c * 512:(c + 1) * 512],
                start=False, stop=(True), skip_group_check=True)

    # ================= gate: yhat = (y + Dp*u) * silu(z), A2A =================
    a2a_i = dram.tile([DI, LT], BF16)
    a2a_o = dram.tile([DI, LT], BF16)
    for dt in range(NDT):
        yh = scanp.tile([128, L], BF16, name=f"yhat{dt}", tag="yhat")
        nc.vector.tensor_tensor(yh[:, :], y_ps[dt][:, :], zs[dt][:, :],
                                op=OP.mult)
        # scatter my 128-ch rows into (8 token-blocks x DIC) layout
        nc.sync.dma_start(
            a2a_i.rearrange("(j c) t -> c j t", c=DIC)[dt * 128:(dt + 1) * 128, :, :],
            yh.rearrange("c (j t) -> c j t", j=NCORES))
    nc.gpsimd.collective_compute("AllToAll", OP.bypass, replica_groups=RG,
                                 ins=[a2a_i.opt()], outs=[a2a_o.opt()])

    # ================= PHASE B (sequence-parallel, my LT tokens) ==========
    yps_stack.close()
    act_stack.close()
    bpool = ctx.enter_context(tc.tile_pool(name="bpool", bufs=1))
    psB = ctx.enter_context(tc.tile_pool(name="psB", bufs=2, space="PSUM"))

    x_tok_sb = bpool.tile([128, NTT, DM], F32)
    nc.sync.dma_start(x_tok_sb[:, :, :],
                      x_tok.rearrange("(tt p) m -> p tt m", p=128))
    n1w_rep = bpool.tile([128, DM], F32)
    s1 = n1w[0:1, :]
    nc.sync.dma_start(n1w_rep[:, :], bass.AP(
        tensor=s1.tensor, offset=s1.offset,
        ap=[[0, 128]] + [list(p) for p in s1.ap[1:]]))
    n2w_rep = bpool.tile([128, DM], F32)
    s2 = n2w[0:1, :]
    nc.sync.dma_start(n2w_rep[:, :], bass.AP(
        tensor=s2.tensor, offset=s2.offset,
        ap=[[0, 128]] + [list(p) for p in s2.ap[1:]]))

    x1_l, scl1_l, xqT_l, fqT_l, scl2_l = [], [], [], [], []

    # ---- out_proj + rmsnorm1 + quant1 ----
    with tc.tile_pool(name="oproj", bufs=1) as opool:
        yfull = opool.tile([128, DI // 128, LT], BF16)
        nc.sync.dma_start(yfull[:, :, :], a2a_o.rearrange("(k p) t -> p k t", p=128))
        for tt in range(NTT):
            hps = psB.tile([128, DM], F32, tag="hps")
            for c in range(DM // 512):
                for k in range(DI // 128):
                    nc.tensor.matmul(
                        hps[:, c * 512:(c + 1) * 512],
                        yfull[:, k, tt * 128:(tt + 1) * 128],
                        woutT_sb[:, k, c * 512:(c + 1) * 512],
                        start=(k == 0), stop=(k == DI // 128 - 1))
            s = bpool.tile([128, DM], F32, name=f"s{tt}", tag=f"s{tt}")
            nc.vector.tensor_tensor(s[:, :], x_tok_sb[:, tt, :], hps[:, :], op=OP.add)
            sq = bpool.tile([128, DM], F32, name=f"sq{tt}", tag="sq")
            ssum = bpool.tile([128, 1], F32, name=f"ssum{tt}", tag="ssum")
            nc.scalar.activation(sq[:, :], s[:, :], AF.Square, accum_out=ssum[:, 0:1])
            v = bpool.tile([128, 1], F32, name=f"v{tt}", tag=f"v{tt}")
            nc.vector.tensor_scalar(v[:, :], ssum[:, :], 1.0 / DM, EPS,
                                    op0=OP.mult, op1=OP.add)
            nc.scalar.activation(v[:, :], v[:, :], AF.Ln)
            nc.scalar.activation(v[:, :], v[:, :], AF.Exp, scale=-0.5)
            x1 = bpool.tile([128, DM], F32, name=f"x1_{tt}", tag=f"x1_{tt}")
            nc.vector.scalar_tensor_tensor(x1[:, :], s[:, :], v[:, 0:1],
                                           n1w_rep[:, :], op0=OP.mult, op1=OP.mult)
            x1_l.append(x1)
            amax = bpool.tile([128, 1], F32, name=f"am{tt}", tag="am")
            nc.vector.tensor_reduce(amax[:, :], x1[:, :], axis=mybir.AxisListType.X,
                                    op=OP.max, apply_absolute_value=True)
            nc.vector.tensor_scalar(amax[:, :], amax[:, :], 1e-5, None, op0=OP.max)
            sc = bpool.tile([128, 1], F32, name=f"sc{tt}", tag="sc")
            nc.vector.reciprocal(sc[:, :], amax[:, :])
            vsc = bpool.tile([128, 1], F32, name=f"vsc{tt}", tag="vsc")
            nc.vector.tensor_tensor(vsc[:, :], sc[:, :], v[:, :], op=OP.mult)
            nc.vector.tensor_scalar(vsc[:, :], vsc[:, :], 127.0, None, op0=OP.mult)
            scl1 = bpool.tile([128, 1], F32, name=f"scl1_{tt}", tag=f"scl1_{tt}")
            nc.vector.tensor_scalar(scl1[:, :], amax[:, :], g1 / 127.0, None,
                                    op0=OP.mult)
            scl1_l.append(scl1)
            q = bpool.tile([128, DM], F32, name=f"q{tt}", tag="q")
            nc.vector.scalar_tensor_tensor(q[:, :], s[:, :], vsc[:, 0:1],
                                           n1w_rep[:, :], op0=OP.mult, op1=OP.mult)
            xq = bpool.tile([128, DM], BF16, name=f"xq{tt}", tag="xq")
            nc.vector.tensor_scalar(xq[:, :], q[:, :], MAGIC, MAGIC,
                                    op0=OP.add, op1=OP.subtract)
            xqT = bpool.tile([128, DM // 128, 128], BF16, name=f"xqT{tt}",
                             tag=f"xqT{tt}")
            nc.sync.dma_start_transpose(xqT[:, :, :], xq[:, :])
            xqT_l.append(xqT)

    # ---- FFN mm1 + gelu + quant2 ----
    with tc.tile_pool(name="ffn1", bufs=1) as f1pool:
        for tt in range(NTT):
            f_sb = f1pool.tile([128, DFF], BF16, name=f"f{tt}", tag="f")
            for jc in range(DFF // 512):
                fps = psB.tile([128, 512], F32, tag="fps")
                for k in range(DM // 128):
                    nc.tensor.matmul(
                        fps[:, :], xqT_l[tt][:, k, :],
                        w1qT_sb[:, k, jc * 512:(jc + 1) * 512],
                        start=(k == 0), stop=(k == DM // 128 - 1))
                nc.scalar.activation(f_sb[:, jc * 512:(jc + 1) * 512], fps[:, :],
                                     AF.Gelu_apprx_tanh, scale=scl1_l[tt][:, 0:1])
            amax2 = bpool.tile([128, 1], F32, name=f"am2{tt}", tag="am2")
            nc.vector.tensor_reduce(amax2[:, :], f_sb[:, :], axis=mybir.AxisListType.X,
                                    op=OP.max, apply_absolute_value=True)
            nc.vector.tensor_scalar(amax2[:, :], amax2[:, :], 1e-5, None, op0=OP.max)
            sc2 = bpool.tile([128, 1], F32, name=f"sc2{tt}", tag="sc2")
            nc.vector.reciprocal(sc2[:, :], amax2[:, :])
            nc.vector.tensor_scalar(sc2[:, :], sc2[:, :], 127.0, None, op0=OP.mult)
            scl2 = bpool.tile([128, 1], F32, name=f"scl2_{tt}", tag=f"scl2_{tt}")
            nc.vector.tensor_scalar(scl2[:, :], amax2[:, :], g2 / 127.0, None,
                                    op0=OP.mult)
            scl2_l.append(scl2)
            q2 = f1pool.tile([128, DFF], BF16, name=f"q2{tt}", tag="q2")
            nc.vector.tensor_scalar(q2[:, :], f_sb[:, :], sc2[:, 0:1], None,
                                    op0=OP.mult)
            fq = f1pool.tile([128, DFF], BF16, name=f"fq{tt}", tag="fq")
            nc.vector.tensor_scalar(fq[:, :], q2[:, :], MAGIC, MAGIC,
                                    op0=OP.add, op1=OP.subtract)
            fqT = bpool.tile([128, DFF // 128, 128], BF16, name=f"fqT{tt}",
                             tag=f"fqT{tt}")
            nc.sync.dma_start_transpose(fqT[:, :, :], fq[:, :])
            fqT_l.append(fqT)

    # ---- FFN mm2 + residual + rmsnorm2 ----
    with tc.tile_pool(name="ffn2", bufs=1) as f2pool:
        for tt in range(NTT):
            o2 = f2pool.tile([128, DM], F32, name=f"o2{tt}", tag="o2")
            for mc in range(DM // 512):
                ops_ = psB.tile([128, 512], F32, tag="ops")
                for k in range(DFF // 128):
                    nc.tensor.matmul(
                        ops_[:, :], fqT_l[tt][:, k, :],
                        w2qT_sb[:, k, mc * 512:(mc + 1) * 512],
                        start=(k == 0), stop=(k == DFF // 128 - 1))
                nc.vector.scalar_tensor_tensor(
                    o2[:, mc * 512:(mc + 1) * 512], ops_[:, :], scl2_l[tt][:, 0:1],
                    x1_l[tt][:, mc * 512:(mc + 1) * 512], op0=OP.mult, op1=OP.add)
            sq2 = f2pool.tile([128, DM], F32, name=f"sq2{tt}", tag="sq2")
            ssum2 = f2pool.tile([128, 1], F32, name=f"ssum2{tt}", tag="ssum2")
            nc.scalar.activation(sq2[:, :], o2[:, :], AF.Square,
                                 accum_out=ssum2[:, 0:1])
            v2 = f2pool.tile([128, 1], F32, name=f"v2{tt}", tag=f"v2{tt}")
            nc.vector.tensor_scalar(v2[:, :], ssum2[:, :], 1.0 / DM, EPS,
                                    op0=OP.mult, op1=OP.add)
            nc.scalar.activation(v2[:, :], v2[:, :], AF.Ln)
            nc.scalar.activation(v2[:, :], v2[:, :], AF.Exp, scale=-0.5)
            ot = f2pool.tile([128, DM], F32, name=f"ot{tt}", tag="ot")
            nc.vector.scalar_tensor_tensor(ot[:, :], o2[:, :], v2[:, 0:1],
                                           n2w_rep[:, :], op0=OP.mult, op1=OP.mult)
            nc.sync.dma_start(out_t[tt * 128:(tt + 1) * 128, :], ot[:, :])


def build_nc(g1, g2, dbg=False, repeat=1):
    from contextlib import ExitStack
    nc = bacc.Bacc("TRN2", target_bir_lowering=False, debug=False,
                   num_devices=NCORES)
    with ExitStack() as ctx:
        tc = ctx.enter_context(tile.TileContext(nc))
        t = _declare(nc)
        for it in range(repeat):
            with ExitStack() as ictx:
                _emit(nc, tc, ictx, g1, g2, t)
    nc.compile()
    return nc


def host_prep(inputs):
    bf = ml_dtypes.bfloat16
    f8 = mybir.dt.np(FP8)
    x = np.asarray(inputs["x"], np.float32)
    x2d = x.reshape(L, DM)
    w_in = np.asarray(inputs["w_in"], np.float32)
    conv_w = np.asarray(inputs["conv_w"], np.float32)
    conv_b = np.asarray(inputs["conv_b"], np.float32)
    w_xproj = np.asarray(inputs["w_xproj"], np.float32)
    w_dt = np.asarray(inputs["w_dt"], np.float32)
    b_dt = np.asarray(inputs["b_dt"], np.float32)
    A_log = np.asarray(inputs["A_log"], np.float32)
    Dp = np.asarray(inputs["Dp"], np.float32)
    w_out = np.asarray(inputs["w_out"], np.float32)
    n1 = np.asarray(inputs["norm1_w"], np.float32)
    n2 = np.asarray(inputs["norm2_w"], np.float32)
    w1 = np.asarray(inputs["ffn_w1"], np.float32)
    w2 = np.asarray(inputs["ffn_w2"], np.float32)
    b1 = np.asarray(inputs["ffn_b1"], np.float32)
    b2 = np.asarray(inputs["ffn_b2"], np.float32)
    assert np.all(b1 == 0.0) and np.all(b2 == 0.0), "nonzero ffn bias unsupported"

    g1 = float(np.maximum(np.mean(np.abs(w1), dtype=np.float32), 1e-5))
    g2 = float(np.maximum(np.mean(np.abs(w2), dtype=np.float32), 1e-5))
    w1q = np.clip(np.rint(w1 / g1), -1.0, 1.0).astype(np.float32)
    w2q = np.clip(np.rint(w2 / g2), -1.0, 1.0).astype(np.float32)

    xT_bf = np.ascontiguousarray(x2d.T).astype(bf)
    woutT_bf = np.ascontiguousarray(w_out.T).astype(bf)
    w1qT_f8 = np.ascontiguousarray(w1q.T).astype(f8)
    w2qT_f8 = np.ascontiguousarray(w2q.T).astype(f8)
    n1r = np.ascontiguousarray(n1.reshape(1, DM))
    n2r = np.ascontiguousarray(n2.reshape(1, DM))
    A = -np.exp(A_log)

    # interleave B/C rows of w_xproj so the kernel broadcasts B_n and C_n in
    # a single DMA: new row 64+2n = B_n (old 64+n), 65+2n = C_n (old 80+n)
    xp_perm = np.concatenate([
        np.arange(DTR),
        np.stack([np.arange(DTR, DTR + DS),
                  np.arange(DTR + DS, DTR + 2 * DS)], axis=1).reshape(-1)])

    in_maps = []
    for c in range(NCORES):
        ch = slice(c * DIC, (c + 1) * DIC)
        w_sel = np.concatenate([w_in[c * DIC:(c + 1) * DIC],
                                w_in[DI + c * DIC:DI + (c + 1) * DIC]], axis=0)
        in_maps.append({
            "xT": xT_bf,
            "x_tok": np.ascontiguousarray(x2d[c * LT:(c + 1) * LT]),
            "winT": np.ascontiguousarray(w_sel.T).astype(bf),
            "convw": np.ascontiguousarray(conv_w[ch, 0, :]),
            "convb": np.ascontiguousarray(conv_b[ch].reshape(-1, 1)),
            "wxpT": np.ascontiguousarray(w_xproj[xp_perm][:, ch].T).astype(bf),
            "wdtT": np.ascontiguousarray(w_dt[ch, :].T).astype(bf),
            "bdt": np.ascontiguousarray(b_dt[ch].reshape(-1, 1)),
            "acol": np.ascontiguousarray(A[ch, :]),
            "dpv": np.ascontiguousarray(Dp[ch].reshape(-1, 1)),
            "woutT": woutT_bf,
            "n1w": n1r,
            "n2w": n2r,
            "w1qT": w1qT_f8,
            "w2qT": w2qT_f8,
        })
    return in_maps, g1, g2


def kernel(**inputs) -> np.ndarray:
    in_maps, g1, g2 = host_prep(inputs)
    key = (round(g1, 10), round(g2, 10))
    if key not in _NC_CACHE:
        _NC_CACHE[key] = build_nc(g1, g2)
    nc = _NC_CACHE[key]
    res = run_bass_kernel_spmd(nc, in_maps, core_ids=list(range(NCORES)))
    out = np.concatenate([res.results[c]["out"] for c in range(NCORES)], axis=0)
    return np.ascontiguousarray(out.reshape(1, L, DM).astype(np.float32))



# revision 42
# speedup vs baseline: 8.3191x; 8.3191x over previous
"""Trainium2 Bass kernel for nn_DecoderLayer (Mamba block + BitNet FFN).

Sharding: channel-parallel mamba (256 ch/core) -> AllReduce (xproj rows) ->
DVE tensor_tensor_scan over (d,n) lanes -> AllToAll (d-shard -> t-shard) ->
sequence-parallel out_proj + rmsnorm + BitNet FFN (host-prequantized ternary
weights as fp8, exact bf16xfp8 matmuls) -> each core emits its 256-token slice.

v2: bf16 conv/activations, fp8 ternary FFN weights, early weight prefetch,
chunked in_proj, redundant clips dropped, repeat=N support for timing.
"""
import numpy as np
import ml_dtypes

try:
    import jax
    jax.config.update("jax_compilation_cache_dir", "/root/jaxcache")
    jax.config.update("jax_persistent_cache_min_compile_time_secs", 1.0)
except Exception:
    pass

import concourse.bass as bass
import concourse.mybir as mybir
import concourse.tile as tile
from concourse import bacc
from concourse.bass_utils import run_bass_kernel_spmd

BF16 = mybir.dt.bfloat16
F32 = mybir.dt.float32
F32R = mybir.dt.float32r
FP8 = mybir.dt.float8e4
AF = mybir.ActivationFunctionType
OP = mybir.AluOpType

L, DM, DI, DS, DC, DTR, DFF = 2048, 1024, 2048, 16, 4, 64, 4096
EPS = 1e-6
NCORES = 8
DIC = DI // NCORES   # 256 channels per core
NDT = DIC // 128     # 2 d-tiles
LT = L // NCORES     # 256 tokens per core
NTT = LT // 128      # 2 token-tiles
MAGIC = 12582912.0   # 1.5*2^23: x+M-M == rint(x) for |x|<2^22

_NC_CACHE = {}


def _declare(nc):
    t = {}
    t["xT"] = nc.dram_tensor("xT", [DM, L], BF16, kind="ExternalInput")
    t["x_tok"] = nc.dram_tensor("x_tok", [LT, DM], F32, kind="ExternalInput")
    t["winT"] = nc.dram_tensor("winT", [DM, 2 * 128 * NDT], BF16, kind="ExternalInput")
    t["convw"] = nc.dram_tensor("convw", [DIC, DC], F32, kind="ExternalInput")
    t["convb"] = nc.dram_tensor("convb", [DIC, 1], F32, kind="ExternalInput")
    t["wxpT"] = nc.dram_tensor("wxpT", [DIC, 96], BF16, kind="ExternalInput")
    t["wdtT"] = nc.dram_tensor("wdtT", [DTR, DIC], BF16, kind="ExternalInput")
    t["bdt"] = nc.dram_tensor("bdt", [DIC, 1], F32, kind="ExternalInput")
    t["acol"] = nc.dram_tensor("acol", [DIC, DS], F32, kind="ExternalInput")
    t["dpv"] = nc.dram_tensor("dpv", [DIC, 1], F32, kind="ExternalInput")
    t["woutT"] = nc.dram_tensor("woutT", [DI, DM], BF16, kind="ExternalInput")
    t["n1w"] = nc.dram_tensor("n1w", [1, DM], F32, kind="ExternalInput")
    t["n2w"] = nc.dram_tensor("n2w", [1, DM], F32, kind="ExternalInput")
    t["w1qT"] = nc.dram_tensor("w1qT", [DM, DFF], FP8, kind="ExternalInput")
    t["w2qT"] = nc.dram_tensor("w2qT", [DFF, DM], FP8, kind="ExternalInput")
    t["out"] = nc.dram_tensor("out", [LT, DM], F32, kind="ExternalOutput")
    return t


def _emit(nc, tc, ctx, g1, g2, t):
    import contextlib
    RG = [list(range(NCORES))]
    xT = t["xT"]; x_tok = t["x_tok"]; winT = t["winT"]; convw = t["convw"]
    convb = t["convb"]; wxpT = t["wxpT"]; wdtT = t["wdtT"]; bdt = t["bdt"]
    acol = t["acol"]; dpv = t["dpv"]; woutT = t["woutT"]; n1w = t["n1w"]
    n2w = t["n2w"]; w1qT = t["w1qT"]; w2qT = t["w2qT"]; out_t = t["out"]

    singles = ctx.enter_context(tc.tile_pool(name="singles", bufs=1))
    dram = ctx.enter_context(tc.tile_pool(name="dram", bufs=1, space="DRAM"))
    wpool = ctx.enter_context(tc.tile_pool(name="wpool", bufs=1))
    psA_stack = contextlib.ExitStack()
    psum_small = psA_stack.enter_context(
        tc.tile_pool(name="psA", bufs=3, space="PSUM"))
    act_stack = contextlib.ExitStack()
    actpool = act_stack.enter_context(tc.tile_pool(name="acts", bufs=1))

    # ---- small per-partition constants
    convw_sb, convb_sb, bdt_sb, acol_sb, dp_sb = [], [], [], [], []
    for dt in range(NDT):
        sl = slice(dt * 128, (dt + 1) * 128)
        t1 = singles.tile([128, DC], F32, name=f"cw{dt}")
        nc.sync.dma_start(t1[:, :], convw[sl, :])
        convw_sb.append(t1)
        t2 = singles.tile([128, 1], F32, name=f"cb{dt}")
        nc.sync.dma_start(t2[:, :], convb[sl, :])
        convb_sb.append(t2)
        t3 = singles.tile([128, 1], F32, name=f"bd{dt}")
        nc.sync.dma_start(t3[:, :], bdt[sl, :])
        bdt_sb.append(t3)
        t4 = singles.tile([128, DS], F32, name=f"ac{dt}")
        nc.sync.dma_start(t4[:, :], acol[sl, :])
        acol_sb.append(t4)
        t5 = singles.tile([128, 1], F32, name=f"dp{dt}")
        nc.sync.dma_start(t5[:, :], dpv[sl, :])
        dp_sb.append(t5)
    wxpT_sb = singles.tile([128, NDT, 96], BF16)
    nc.sync.dma_start(wxpT_sb[:, :, :],
                      wxpT.rearrange("(k p) m -> p k m", p=128))
    wdtT_sb = singles.tile([DTR, DIC], BF16)
    nc.sync.dma_start(wdtT_sb[:, :], wdtT[:, :])
    ident_bf = singles.tile([128, 128], BF16)
    from concourse.masks import make_identity
    make_identity(nc, ident_bf[:, :])
    dpdiag = []
    for dt in range(NDT):
        d = singles.tile([128, 128], BF16, name=f"dpd{dt}")
        nc.vector.tensor_scalar_mul(d[:, :], ident_bf[:, :], dp_sb[dt][:, 0:1])
        dpdiag.append(d)

    # ================= PHASE A: in_proj (channel-parallel) =================
    conv_stack = contextlib.ExitStack()
    convpool = conv_stack.enter_context(tc.tile_pool(name="convp", bufs=1))
    NXC = 4                    # xT chunks along L
    XC = L // NXC
    with tc.tile_pool(name="init", bufs=1) as init_pool:
        winT_sb = init_pool.tile([128, 8, 2 * 128 * NDT], BF16)
        nc.sync.dma_start(winT_sb[:, :, :],
                          winT.rearrange("(k p) m -> p k m", p=128))
        xT_re = xT.rearrange("(k p) l -> p k l", p=128)
        xT_c = []
        for c in range(NXC):
            xc = init_pool.tile([128, 8, XC], BF16, name=f"xc{c}")
            nc.sync.dma_start(xc[:, :, :], xT_re[:, :, c * XC:(c + 1) * XC])
            xT_c.append(xc)

        # ---- prefetch phase-B weights (queue behind critical loads)
        woutT_sb = wpool.tile([128, DI // 128, DM], BF16)
        nc.sync.dma_start(woutT_sb[:, :, :],
                          woutT.rearrange("(k p) m -> p k m", p=128))
        w1qT_sb = wpool.tile([128, 8, DFF], FP8)
        nc.sync.dma_start(w1qT_sb[:, :, :],
                          w1qT.rearrange("(k p) j -> p k j", p=128))
        w2qT_sb = wpool.tile([128, DFF // 128, DM], FP8)
        nc.sync.dma_start(w2qT_sb[:, :, :],
                          w2qT.rearrange("(k p) m -> p k m", p=128))

        u_pad, zs = [], []
        for dt in range(NDT):
            up = convpool.tile([128, L + 3], BF16, name=f"upad{dt}")
            nc.vector.memset(up[:, 0:3], 0.0)
            u_pad.append(up)
            zs.append(actpool.tile([128, L], BF16, name=f"zs{dt}"))

        # m-tiles: 0..NDT-1 are u chunks, NDT..2*NDT-1 are z chunks
        for c in range(NXC):
            for mt in range(2 * NDT):
                for cc in range(XC // 512):
                    ps = psum_small.tile([128, 512], F32, tag="psA")
                    lo = c * XC + cc * 512
                    for k in range(8):
                        nc.tensor.matmul(
                            ps[:, :],
                            winT_sb[:, k, mt * 128:(mt + 1) * 128],
                            xT_c[c][:, k, cc * 512:(cc + 1) * 512],
                            start=(k == 0), stop=(k == 7))
                    if mt < NDT:
                        nc.scalar.copy(u_pad[mt][:, 3 + lo: 3 + lo + 512],
                                       ps[:, :])
                    else:
                        nc.scalar.activation(
                            zs[mt - NDT][:, lo: lo + 512], ps[:, :], AF.Silu)

    # ================= conv + silu (bf16 chain) =================
    u_act = []
    for dt in range(NDT):
        ca = convpool.tile([128, L], BF16, name=f"cva{dt}", tag="cva")
        cb = convpool.tile([128, L], BF16, name=f"cvb{dt}", tag="cvb")
        nc.vector.tensor_scalar_mul(ca[:, :], u_pad[dt][:, 0:L],
                                    convw_sb[dt][:, 0:1])
        nc.vector.scalar_tensor_tensor(
            cb[:, :], u_pad[dt][:, 1:L + 1], convw_sb[dt][:, 1:2], ca[:, :],
            op0=OP.mult, op1=OP.add)
        nc.vector.scalar_tensor_tensor(
            ca[:, :], u_pad[dt][:, 2:L + 2], convw_sb[dt][:, 2:3], cb[:, :],
            op0=OP.mult, op1=OP.add)
        nc.vector.scalar_tensor_tensor(
            cb[:, :], u_pad[dt][:, 3:L + 3], convw_sb[dt][:, 3:4], ca[:, :],
            op0=OP.mult, op1=OP.add)
        ua = actpool.tile([128, L], BF16, name=f"uact{dt}")
        nc.scalar.activation(ua[:, :], cb[:, :], AF.Silu,
                             bias=convb_sb[dt][:, 0:1])
        u_act.append(ua)
    conv_stack.close()


    # ================= xproj partial + AllReduce =================
    xp_stack = contextlib.ExitStack()
    xppool = xp_stack.enter_context(tc.tile_pool(name="xpp", bufs=1))
    dbl_loc = xppool.tile([96, L], BF16)
    for c in range(L // 512):
        ps = psum_small.tile([96, 512], F32, tag="psA")
        for kt in range(NDT):
            nc.tensor.matmul(
                ps[:, :],
                wxpT_sb[:, kt, :],
                u_act[kt][:, c * 512:(c + 1) * 512],
                start=(kt == 0), stop=(kt == NDT - 1))
        nc.scalar.copy(dbl_loc[:, c * 512:(c + 1) * 512], ps[:, :])

    # bf16 AllReduce (halves the collective payload); B/C broadcasts read the
    # AR output in DRAM directly, no bounce copy needed
    ar_i = dram.tile([96, L], BF16)
    ar_o = dram.tile([96, L], BF16, addr_space="Shared")
    nc.sync.dma_start(ar_i[:, :], dbl_loc[:, :])
    nc.gpsimd.collective_compute("AllReduce", OP.add, replica_groups=RG,
                                 ins=[ar_i.opt()], outs=[ar_o.opt()])
    dbl_bf = xppool.tile([64, L], BF16)
    nc.sync.dma_start(dbl_bf[:, :], ar_o[0:DTR, :])
    bcb = ar_o

    # ================= delta = softplus(wdt @ dt + bdt) =================
    # all Exp ops batched before the Ln ops: fewer act-table reloads
    delta = []
    for dt in range(NDT):
        dl = actpool.tile([128, L], BF16, name=f"delta{dt}")
        for c in range(L // 512):
            ps = psum_small.tile([128, 512], F32, tag="psA")
            nc.tensor.matmul(
                ps[:, :],
                wdtT_sb[:, dt * 128:(dt + 1) * 128],
                dbl_bf[0:DTR, c * 512:(c + 1) * 512],
                start=True, stop=True)
            # exp(x + bdt) from PSUM, then ln(1+e) in-place later
            nc.scalar.activation(dl[:, c * 512:(c + 1) * 512], ps[:, :],
                                 AF.Exp, bias=bdt_sb[dt][:, 0:1])
        delta.append(dl)
    for dt in range(NDT):
        nc.scalar.activation(delta[dt][:, :], delta[dt][:, :], AF.Ln, bias=1.0)

    xp_stack.close()
    # delta*u in bf16 for the scan input product
    du_bf = []
    for dt in range(NDT):
        db = actpool.tile([128, L], BF16, name=f"dubf{dt}")
        nc.vector.tensor_tensor(db[:, :], delta[dt][:, :], u_act[dt][:, :],
                                op=OP.mult)
        du_bf.append(db)

    # ================= scan over n (16 states) =================
    psA_stack.close()
    yps_stack = contextlib.ExitStack()
    y_ps_pool = yps_stack.enter_context(
        tc.tile_pool(name="yps", bufs=1, space="PSUM"))
    y_ps = [y_ps_pool.tile([128, L], F32, name=f"yps{dt}") for dt in range(NDT)]

    scanp = act_stack.enter_context(tc.tile_pool(name="scanp", bufs=2))
    repp = act_stack.enter_context(tc.tile_pool(name="repp", bufs=3))
    for n in range(DS):
        brep = repp.tile([128, L], BF16, name=f"brep{n}", tag="brep")
        b_src = bcb[DTR + n:DTR + n + 1, :]
        nc.sync.dma_start(brep[:, :], bass.AP(
            tensor=b_src.tensor, offset=b_src.offset,
            ap=[[0, 128]] + [list(p) for p in b_src.ap[1:]]))
        crep = repp.tile([128, L], BF16, name=f"crep{n}", tag="crep")
        c_src = bcb[DTR + DS + n:DTR + DS + n + 1, :]
        nc.sync.dma_start(crep[:, :], bass.AP(
            tensor=c_src.tensor, offset=c_src.offset,
            ap=[[0, 128]] + [list(p) for p in c_src.ap[1:]]))
        for dt in range(NDT):
            dA = scanp.tile([128, L], BF16, name=f"dA{n}_{dt}", tag="dA")
            nc.scalar.activation(dA[:, :], delta[dt][:, :], AF.Exp,
                                 scale=acol_sb[dt][:, n:n + 1])
            dBu = scanp.tile([128, L], BF16, name=f"dBu{n}_{dt}", tag="dBu")
            nc.vector.tensor_tensor(dBu[:, :], du_bf[dt][:, :], brep[:, :],
                                    op=OP.mult)
            h = scanp.tile([128, L], BF16, name=f"h{n}_{dt}", tag="h")
            nc.vector.tensor_tensor_scan(h[:, :], dA[:, :], dBu[:, :], 0.0,
                                         OP.mult, OP.add)
            yt = scanp.tile([128, L], BF16, name=f"yt{n}_{dt}", tag="yt")
            nc.vector.tensor_tensor(yt[:, :], h[:, :], crep[:, :], op=OP.mult)
            for c in range(L // 512):
                nc.tensor.matmul(
                    y_ps[dt][:, c * 512:(c + 1) * 512],
                    ident_bf[:, :],
                    yt[:, c * 512:(c + 1) * 512],
                    start=(n == 0), stop=False,
                    skip_group_check=True)
    # final accumulation term: Dp * u via a diagonal stationary matrix
    for dt in range(NDT):
        for c in range(L // 512):
            nc.tensor.matmul(
                y_ps[dt][:, c * 512:(c + 1) * 512],
                dpdiag[dt][:, :],
                u_act[dt][:, c * 512:(c + 1) * 512],
                start=False, stop=(True), skip_group_check=True)

    # ================= gate: yhat = (y + Dp*u) * silu(z), A2A =================
    a2a_i = dram.tile([DI, LT], BF16)
    a2a_o = dram.tile([DI, LT], BF16)
    for dt in range(NDT):
        yh = scanp.tile([128, L], BF16, name=f"yhat{dt}", tag="yhat")
        nc.vector.tensor_tensor(yh[:, :], y_ps[dt][:, :], zs[dt][:, :],
                                op=OP.mult)
        # scatter my 128-ch rows into (8 token-blocks x DIC) layout
        nc.sync.dma_start(
            a2a_i.rearrange("(j c) t -> c j t", c=DIC)[dt * 128:(dt + 1) * 128, :, :],
            yh.rearrange("c (j t) -> c j t", j=NCORES))
    nc.gpsimd.collective_compute("AllToAll", OP.bypass, replica_groups=RG,
                                 ins=[a2a_i.opt()], outs=[a2a_o.opt()])

    # ================= PHASE B (sequence-parallel, my LT tokens) ==========
    yps_stack.close()
    act_stack.close()
    bpool = ctx.enter_context(tc.tile_pool(name="bpool", bufs=1))
    psB = ctx.enter_context(tc.tile_pool(name="psB", bufs=2, space="PSUM"))

    x_tok_sb = bpool.tile([128, NTT, DM], F32)
    nc.sync.dma_start(x_tok_sb[:, :, :],
                      x_tok.rearrange("(tt p) m -> p tt m", p=128))
    n1w_rep = bpool.tile([128, DM], F32)
    s1 = n1w[0:1, :]
    nc.sync.dma_start(n1w_rep[:, :], bass.AP(
        tensor=s1.tensor, offset=s1.offset,
        ap=[[0, 128]] + [list(p) for p in s1.ap[1:]]))
    n2w_rep = bpool.tile([128, DM], F32)
    s2 = n2w[0:1, :]
    nc.sync.dma_start(n2w_rep[:, :], bass.AP(
        tensor=s2.tensor, offset=s2.offset,
        ap=[[0, 128]] + [list(p) for p in s2.ap[1:]]))

    x1_l, scl1_l, xqT_l, fqT_l, scl2_l = [], [], [], [], []

    # ---- out_proj + rmsnorm1 + quant1 ----
    with tc.tile_pool(name="oproj", bufs=1) as opool:
        yfull = opool.tile([128, DI // 128, LT], BF16)
        nc.sync.dma_start(yfull[:, :, :], a2a_o.rearrange("(k p) t -> p k t", p=128))
        for tt in range(NTT):
            hps = psB.tile([128, DM], F32, tag="hps")
            for c in range(DM // 512):
                for k in range(DI // 128):
                    nc.tensor.matmul(
                        hps[:, c * 512:(c + 1) * 512],
                        yfull[:, k, tt * 128:(tt + 1) * 128],
                        woutT_sb[:, k, c * 512:(c + 1) * 512],
                        start=(k == 0), stop=(k == DI // 128 - 1))
            s = bpool.tile([128, DM], F32, name=f"s{tt}", tag=f"s{tt}")
            nc.vector.tensor_tensor(s[:, :], x_tok_sb[:, tt, :], hps[:, :], op=OP.add)
            sq = bpool.tile([128, DM], F32, name=f"sq{tt}", tag="sq")
            ssum = bpool.tile([128, 1], F32, name=f"ssum{tt}", tag="ssum")
            nc.scalar.activation(sq[:, :], s[:, :], AF.Square, accum_out=ssum[:, 0:1])
            v = bpool.tile([128, 1], F32, name=f"v{tt}", tag=f"v{tt}")
            nc.vector.tensor_scalar(v[:, :], ssum[:, :], 1.0 / DM, EPS,
                                    op0=OP.mult, op1=OP.add)
            nc.scalar.activation(v[:, :], v[:, :], AF.Ln)
            nc.scalar.activation(v[:, :], v[:, :], AF.Exp, scale=-0.5)
            x1 = bpool.tile([128, DM], F32, name=f"x1_{tt}", tag=f"x1_{tt}")
            nc.vector.scalar_tensor_tensor(x1[:, :], s[:, :], v[:, 0:1],
                                           n1w_rep[:, :], op0=OP.mult, op1=OP.mult)
            x1_l.append(x1)
            amax = bpool.tile([128, 1], F32, name=f"am{tt}", tag="am")
            nc.vector.tensor_reduce(amax[:, :], x1[:, :], axis=mybir.AxisListType.X,
                                    op=OP.max, apply_absolute_value=True)
            nc.vector.tensor_scalar(amax[:, :], amax[:, :], 1e-5, None, op0=OP.max)
            sc = bpool.tile([128, 1], F32, name=f"sc{tt}", tag="sc")
            nc.vector.reciprocal(sc[:, :], amax[:, :])
            vsc = bpool.tile([128, 1], F32, name=f"vsc{tt}", tag="vsc")
            nc.vector.tensor_tensor(vsc[:, :], sc[:, :], v[:, :], op=OP.mult)
            nc.vector.tensor_scalar(vsc[:, :], vsc[:, :], 127.0, None, op0=OP.mult)
            scl1 = bpool.tile([128, 1], F32, name=f"scl1_{tt}", tag=f"scl1_{tt}")
            nc.vector.tensor_scalar(scl1[:, :], amax[:, :], g1 / 127.0, None,
                                    op0=OP.mult)
            scl1_l.append(scl1)
            q = bpool.tile([128, DM], F32, name=f"q{tt}", tag="q")
            nc.vector.scalar_tensor_tensor(q[:, :], s[:, :], vsc[:, 0:1],
                                           n1w_rep[:, :], op0=OP.mult, op1=OP.mult)
            xq = bpool.tile([128, DM], BF16, name=f"xq{tt}", tag="xq")
            nc.vector.tensor_scalar(xq[:, :], q[:, :], MAGIC, MAGIC,
                                    op0=OP.add, op1=OP.subtract)
            xqT = bpool.tile([128, DM // 128, 128], BF16, name=f"xqT{tt}",
                             tag=f"xqT{tt}")
            nc.sync.dma_start_transpose(xqT[:, :, :], xq[:, :])
            xqT_l.append(xqT)

    # ---- FFN mm1 + gelu + quant2 ----
    with tc.tile_pool(name="ffn1", bufs=1) as f1pool:
        for tt in range(NTT):
            f_sb = f1pool.tile([128, DFF], BF16, name=f"f{tt}", tag="f")
            for jc in range(DFF // 512):
                fps = psB.tile([128, 512], F32, tag="fps")
                for k in range(DM // 128):
                    nc.tensor.matmul(
                        fps[:, :], xqT_l[tt][:, k, :],
                        w1qT_sb[:, k, jc * 512:(jc + 1) * 512],
                        start=(k == 0), stop=(k == DM // 128 - 1))
                nc.scalar.activation(f_sb[:, jc * 512:(jc + 1) * 512], fps[:, :],
                                     AF.Gelu_apprx_tanh, scale=scl1_l[tt][:, 0:1])
            amax2 = bpool.tile([128, 1], F32, name=f"am2{tt}", tag="am2")
            nc.vector.tensor_reduce(amax2[:, :], f_sb[:, :], axis=mybir.AxisListType.X,
                                    op=OP.max, apply_absolute_value=True)
            nc.vector.tensor_scalar(amax2[:, :], amax2[:, :], 1e-5, None, op0=OP.max)
            sc2 = bpool.tile([128, 1], F32, name=f"sc2{tt}", tag="sc2")
            nc.vector.reciprocal(sc2[:, :], amax2[:, :])
            nc.vector.tensor_scalar(sc2[:, :], sc2[:, :], 127.0, None, op0=OP.mult)
            scl2 = bpool.tile([128, 1], F32, name=f"scl2_{tt}", tag=f"scl2_{tt}")
            nc.vector.tensor_scalar(scl2[:, :], amax2[:, :], g2 / 127.0, None,
                                    op0=OP.mult)
            scl2_l.append(scl2)
            q2 = f1pool.tile([128, DFF], BF16, name=f"q2{tt}", tag="q2")
            nc.vector.tensor_scalar(q2[:, :], f_sb[:, :], sc2[:, 0:1], None,
                                    op0=OP.mult)
            fq = f1pool.tile([128, DFF], BF16, name=f"fq{tt}", tag="fq")
            nc.vector.tensor_scalar(fq[:, :], q2[:, :], MAGIC, MAGIC,
                                    op0=OP.add, op1=OP.subtract)
            fqT = bpool.tile([128, DFF // 128, 128], BF16, name=f"fqT{tt}",
                             tag=f"fqT{tt}")
            nc.sync.dma_start_transpose(fqT[:, :, :], fq[:, :])
            fqT_l.append(fqT)

    # ---- FFN mm2 + residual + rmsnorm2 ----
    with tc.tile_pool(name="ffn2", bufs=1) as f2pool:
        for tt in range(NTT):
            o2 = f2pool.tile([128, DM], F32, name=f"o2{tt}", tag="o2")
            for mc in range(DM // 512):
                ops_ = psB.tile([128, 512], F32, tag="ops")
                for k in range(DFF // 128):
                    nc.tensor.matmul(
                        ops_[:, :], fqT_l[tt][:, k, :],
                        w2qT_sb[:, k, mc * 512:(mc + 1) * 512],
                        start=(k == 0), stop=(k == DFF // 128 - 1))
                nc.vector.scalar_tensor_tensor(
                    o2[:, mc * 512:(mc + 1) * 512], ops_[:, :], scl2_l[tt][:, 0:1],
                    x1_l[tt][:, mc * 512:(mc + 1) * 512], op0=OP.mult, op1=OP.add)
            sq2 = f2pool.tile([128, DM], F32, name=f"sq2{tt}", tag="sq2")
            ssum2 = f2pool.tile([128, 1], F32, name=f"ssum2{tt}", tag="ssum2")
            nc.scalar.activation(sq2[:, :], o2[:, :], AF.Square,
                                 accum_out=ssum2[:, 0:1])
            v2 = f2pool.tile([128, 1], F32, name=f"v2{tt}", tag=f"v2{tt}")
            nc.vector.tensor_scalar(v2[:, :], ssum2[:, :], 1.0 / DM, EPS,
                                    op0=OP.mult, op1=OP.add)
            nc.scalar.activation(v2[:, :], v2[:, :], AF.Ln)
            nc.scalar.activation(v2[:, :], v2[:, :], AF.Exp, scale=-0.5)
            ot = f2pool.tile([128, DM], F32, name=f"ot{tt}", tag="ot")
            nc.vector.scalar_tensor_tensor(ot[:, :], o2[:, :], v2[:, 0:1],
                                           n2w_rep[:, :], op0=OP.mult, op1=OP.mult)
            nc.sync.dma_start(out_t[tt * 128:(tt + 1) * 128, :], ot[:, :])


def build_nc(g1, g2, dbg=False, repeat=1):
    from contextlib import ExitStack
    nc = bacc.Bacc("TRN2", target_bir_lowering=False, debug=False,
                   num_devices=NCORES)
    with ExitStack() as ctx:
        tc = ctx.enter_context(tile.TileContext(nc))
        t = _declare(nc)
        for it in range(repeat):
            with ExitStack() as ictx:
                _emit(nc, tc, ictx, g1, g2, t)
    nc.compile()
    return nc


def host_prep(inputs):
    bf = ml_dtypes.bfloat16
    f8 = mybir.dt.np(FP8)
    x = np.asarray(inputs["x"], np.float32)
    x2d = x.reshape(L, DM)
    w_in = np.asarray(inputs["w_in"], np.float32)
    conv_w = np.asarray(inputs["conv_w"], np.float32)
    conv_b = np.asarray(inputs["conv_b"], np.float32)
    w_xproj = np.asarray(inputs["w_xproj"], np.float32)
    w_dt = np.asarray(inputs["w_dt"], np.float32)
    b_dt = np.asarray(inputs["b_dt"], np.float32)
    A_log = np.asarray(inputs["A_log"], np.float32)
    Dp = np.asarray(inputs["Dp"], np.float32)
    w_out = np.asarray(inputs["w_out"], np.float32)
    n1 = np.asarray(inputs["norm1_w"], np.float32)
    n2 = np.asarray(inputs["norm2_w"], np.float32)
    w1 = np.asarray(inputs["ffn_w1"], np.float32)
    w2 = np.asarray(inputs["ffn_w2"], np.float32)
    b1 = np.asarray(inputs["ffn_b1"], np.float32)
    b2 = np.asarray(inputs["ffn_b2"], np.float32)
    assert np.all(b1 == 0.0) and np.all(b2 == 0.0), "nonzero ffn bias unsupported"

    g1 = float(np.maximum(np.mean(np.abs(w1), dtype=np.float32), 1e-5))
    g2 = float(np.maximum(np.mean(np.abs(w2), dtype=np.float32), 1e-5))
    w1q = np.clip(np.rint(w1 / g1), -1.0, 1.0).astype(np.float32)
    w2q = np.clip(np.rint(w2 / g2), -1.0, 1.0).astype(np.float32)

    xT_bf = np.ascontiguousarray(x2d.T).astype(bf)
    woutT_bf = np.ascontiguousarray(w_out.T).astype(bf)
    w1qT_f8 = np.ascontiguousarray(w1q.T).astype(f8)
    w2qT_f8 = np.ascontiguousarray(w2q.T).astype(f8)
    n1r = np.ascontiguousarray(n1.reshape(1, DM))
    n2r = np.ascontiguousarray(n2.reshape(1, DM))
    A = -np.exp(A_log)

    in_maps = []
    for c in range(NCORES):
        ch = slice(c * DIC, (c + 1) * DIC)
        w_sel = np.concatenate([w_in[c * DIC:(c + 1) * DIC],
                                w_in[DI + c * DIC:DI + (c + 1) * DIC]], axis=0)
        in_maps.append({
            "xT": xT_bf,
            "x_tok": np.ascontiguousarray(x2d[c * LT:(c + 1) * LT]),
            "winT": np.ascontiguousarray(w_sel.T).astype(bf),
            "convw": np.ascontiguousarray(conv_w[ch, 0, :]),
            "convb": np.ascontiguousarray(conv_b[ch].reshape(-1, 1)),
            "wxpT": np.ascontiguousarray(w_xproj[:, ch].T).astype(bf),
            "wdtT": np.ascontiguousarray(w_dt[ch, :].T).astype(bf),
            "bdt": np.ascontiguousarray(b_dt[ch].reshape(-1, 1)),
            "acol": np.ascontiguousarray(A[ch, :]),
            "dpv": np.ascontiguousarray(Dp[ch].reshape(-1, 1)),
            "woutT": woutT_bf,
            "n1w": n1r,
            "n2w": n2r,
            "w1qT": w1qT_f8,
            "w2qT": w2qT_f8,
        })
    return in_maps, g1, g2


def kernel(**inputs) -> np.ndarray:
    in_maps, g1, g2 = host_prep(inputs)
    key = (round(g1, 10), round(g2, 10))
    if key not in _NC_CACHE:
        _NC_CACHE[key] = build_nc(g1, g2)
    nc = _NC_CACHE[key]
    res = run_bass_kernel_spmd(nc, in_maps, core_ids=list(range(NCORES)))
    out = np.concatenate([res.results[c]["out"] for c in range(NCORES)], axis=0)
    return np.ascontiguousarray(out.reshape(1, L, DM).astype(np.float32))

